# revision 1
# baseline (speedup 1.0000x reference)
"""Int8 Baichuan layer on 8 trn2 cores.

Design:
  Program 1 (heads-TP): every core normalizes+quantizes all tokens (ln1),
    computes q/k/v for ITS 4 heads (weights sharded on output dim), RoPE,
    causal attention in fp32, quantizes attn output -> aq [512 dims, 4096 tok].
  Host: redistributes aq from head-sharding to token-sharding.
  Program 2 (token-split): each core takes 512 tokens with full attn dims:
    o-proj + residual, ln2, gate/up/silu/mul/quant, down-proj + residual.
  Host: concatenates per-core [H, 512] outputs, transposes to [B,S,H].

All "int8" matmuls run as bf16 (exact for |int|<=127, fp32 PSUM accum).
Attention matmuls run fp32. Quantization = magic-number rint + clamp on DVE.
"""
import sys
sys.path.insert(0, "/opt/trn_rl_repo")
import math
import os
import numpy as np
import ml_dtypes

import concourse.bass as bass
import concourse.bacc as bacc
import concourse.tile as tile
from concourse import mybir
from concourse.bass_utils import run_bass_kernel_spmd

F32 = mybir.dt.float32
BF16 = mybir.dt.bfloat16
AF = mybir.ActivationFunctionType
ALU = mybir.AluOpType
MAGIC = float(np.float32(1.5 * 2 ** 23))
NC_ = 8
EPS = 1e-6


def _ceil_div(a, b):
    return (a + b - 1) // b


def build_prog1(B, S, H, NH, HD, TPC, sched, n_maskp, qscale, oqs):
    """ln1 + qkv + rope + attention + quant. Per-core SPMD program."""
    T = B * S
    KH = H // 128          # h k-tiles
    HPC = NH // NC_        # heads per core
    DPC = HPC * HD         # q dims per core
    TB = T // TPC          # token blocks
    QPB = S // TPC         # q blocks per batch
    NKT = S // 128         # kt tiles per batch

    nc = bacc.Bacc("TRN2", target_bir_lowering=False, debug=False, num_devices=NC_)
    hid = nc.dram_tensor("hid_T", [H, T], F32, kind="ExternalInput")
    qk_w = nc.dram_tensor("qk_wT", [H, 2 * DPC], BF16, kind="ExternalInput")
    v_w = nc.dram_tensor("v_wT", [H, DPC], BF16, kind="ExternalInput")
    cosq = nc.dram_tensor("cosq", [HD, S], F32, kind="ExternalInput")
    sinq = nc.dram_tensor("sinq", [HD, S], F32, kind="ExternalInput")
    cosk = nc.dram_tensor("cosk", [HD, S], F32, kind="ExternalInput")
    sink = nc.dram_tensor("sink", [HD, S], F32, kind="ExternalInput")
    rms1 = nc.dram_tensor("rms1", [128, KH], F32, kind="ExternalInput")
    maskp = nc.dram_tensor("maskp", [n_maskp, 128, TPC], F32, kind="ExternalInput")
    aq_out = nc.dram_tensor("aq_out", [DPC, T], BF16, kind="ExternalOutput")

    qT_sp = nc.dram_tensor("qT_sp", [DPC, T], F32)
    kT_sp = nc.dram_tensor("kT_sp", [DPC, T], F32)
    v_sp = nc.dram_tensor("v_sp", [T, DPC], F32)

    with tile.TileContext(nc) as tc:
        import contextlib
        with contextlib.ExitStack() as top:
            const = top.enter_context(tc.tile_pool(name="const", bufs=1))
            ones = const.tile([128, 1], F32, tag="ones")
            nc.vector.memset(ones[:], 1.0)
            ones1 = const.tile([1, 128], F32, tag="ones1")
            nc.vector.memset(ones1[:], 1.0)
            rms1_t = const.tile([128, KH], F32, tag="rms1t")
            nc.sync.dma_start(rms1_t[:], rms1.ap())
            mask_t = const.tile([128, n_maskp, TPC], F32, tag="maskt")
            for i in range(n_maskp):
                nc.sync.dma_start(mask_t[:, i, :], maskp.ap()[i, :, :])
            vw_t = const.tile([128, KH, DPC], BF16, tag="vwt")
            for k in range(KH):
                nc.sync.dma_start(vw_t[:, k, :], qk_w.ap()[k * 128:(k + 1) * 128, 0:0] if False else v_w.ap()[k * 128:(k + 1) * 128, :])

            # ---------------- phase A: ln1 + qkv + rope ----------------
            with contextlib.ExitStack() as ph:
                xsq = ph.enter_context(tc.tile_pool(name="xsq", bufs=4))
                xln = ph.enter_context(tc.tile_pool(name="xln", bufs=4))
                lnq = ph.enter_context(tc.tile_pool(name="lnq", bufs=2))
                wst = ph.enter_context(tc.tile_pool(name="wst", bufs=4))
                rop = ph.enter_context(tc.tile_pool(name="rop", bufs=2))
                sm = ph.enter_context(tc.tile_pool(name="sm", bufs=1))
                ps_ss = ph.enter_context(tc.tile_pool(name="ps_ss", bufs=1, space="PSUM"))
                ps_bc = ph.enter_context(tc.tile_pool(name="ps_bc", bufs=1, space="PSUM"))
                ps_mm = ph.enter_context(tc.tile_pool(name="ps_mm", bufs=3, space="PSUM"))

                for tb in range(TB):
                    b = tb // QPB
                    s0 = (tb % QPB) * TPC
                    # sum of squares over H via ones-matmul on x^2
                    p_ss = ps_ss.tile([1, TPC], F32, tag="pss")
                    for k in range(KH):
                        xt = xsq.tile([128, TPC], F32, tag="xt")
                        nc.sync.dma_start(xt[:], hid.ap()[k * 128:(k + 1) * 128, tb * TPC:(tb + 1) * TPC])
                        zt = xsq.tile([128, TPC], F32, tag="zt")
                        nc.scalar.activation(zt[:], xt[:], AF.Square)
                        nc.tensor.matmul(p_ss[:], ones[:], zt[:], start=(k == 0), stop=(k == KH - 1))
                    # rsqrt(mean + eps) with 2 Newton steps
                    vt = sm.tile([1, TPC], F32, tag="vt")
                    nc.vector.tensor_scalar(vt[:], p_ss[:], 1.0 / H, EPS, op0=ALU.mult, op1=ALU.add)
                    rt = sm.tile([1, TPC], F32, tag="rt")
                    nc.vector.reciprocal(rt[:], vt[:])
                    yt = sm.tile([1, TPC], F32, tag="yt")
                    nc.scalar.activation(yt[:], rt[:], AF.Sqrt)
                    for it_ in range(2):
                        u = sm.tile([1, TPC], F32, tag=f"u_nw{it_}", name=f"u_nw{it_}")
                        nc.vector.tensor_tensor(u[:], yt[:], yt[:], op=ALU.mult)
                        nc.vector.tensor_tensor(u[:], u[:], vt[:], op=ALU.mult)
                        nc.vector.tensor_scalar(u[:], u[:], -0.5, 1.5, op0=ALU.mult, op1=ALU.add)
                        y2 = sm.tile([1, TPC], F32, tag=f"y2_nw{it_}", name=f"y2_nw{it_}")
                        nc.vector.tensor_tensor(y2[:], yt[:], u[:], op=ALU.mult)
                        yt = y2
                    p_bc = ps_bc.tile([128, TPC], F32, tag="pbc")
                    nc.tensor.matmul(p_bc[:], ones1[:], yt[:], start=True, stop=True)
                    rs_bc = sm.tile([128, TPC], F32, tag="rsbc")
                    nc.vector.tensor_copy(rs_bc[:], p_bc[:])
                    # ln1q tiles (bf16 ints)
                    lt = []
                    for k in range(KH):
                        x2 = xln.tile([128, TPC], F32, tag="x2")
                        nc.sync.dma_start(x2[:], hid.ap()[k * 128:(k + 1) * 128, tb * TPC:(tb + 1) * TPC])
                        f = xln.tile([128, TPC], F32, tag="f")
                        nc.vector.scalar_tensor_tensor(f[:], x2[:], rms1_t[:, k:k + 1], rs_bc[:], op0=ALU.mult, op1=ALU.mult)
                        nc.vector.tensor_scalar(f[:], f[:], MAGIC, MAGIC, op0=ALU.add, op1=ALU.subtract)
                        q8 = lnq.tile([128, TPC], BF16, tag=f"lnq{k}")
                        nc.vector.tensor_scalar(q8[:], f[:], 127.0, -127.0, op0=ALU.min, op1=ALU.max)
                        lt.append(q8)
                    # q/k matmuls + rope
                    cq = rop.tile([HD, TPC], F32, tag="cq")
                    nc.sync.dma_start(cq[:], cosq.ap()[:, s0:s0 + TPC])
                    sq = rop.tile([HD, TPC], F32, tag="sq")
                    nc.sync.dma_start(sq[:], sinq.ap()[:, s0:s0 + TPC])
                    ck = rop.tile([HD, TPC], F32, tag="ck")
                    nc.sync.dma_start(ck[:], cosk.ap()[:, s0:s0 + TPC])
                    sk = rop.tile([HD, TPC], F32, tag="sk")
                    nc.sync.dma_start(sk[:], sink.ap()[:, s0:s0 + TPC])
                    for m in range(2 * HPC):
                        is_q = m < HPC
                        p_qk = ps_mm.tile([128, TPC], F32, tag="pqk")
                        for k in range(KH):
                            wt = wst.tile([128, 128], BF16, tag="wqk")
                            nc.sync.dma_start(wt[:], qk_w.ap()[k * 128:(k + 1) * 128, m * 128:(m + 1) * 128])
                            nc.tensor.matmul(p_qk[:], wt[:], lt[k][:], start=(k == 0), stop=(k == KH - 1))
                        ctab = cq if is_q else ck
                        stab = sq if is_q else sk
                        t1 = rop.tile([HD, TPC], F32, tag="t1")
                        nc.vector.tensor_tensor(t1[:], p_qk[:], ctab[:], op=ALU.mult)
                        qs = rop.tile([HD, TPC], F32, tag="qs")
                        nc.vector.tensor_copy(qs[:], p_qk[:])
                        qsh = rop.tile([HD, TPC], F32, tag="qsh")
                        hh = HD // 2
                        nc.sync.dma_start(qsh[0:hh, :], qs[hh:HD, :])
                        nc.sync.dma_start(qsh[hh:HD, :], qs[0:hh, :])
                        t2 = rop.tile([HD, TPC], F32, tag="t2")
                        nc.vector.tensor_tensor(t2[:], qsh[:], stab[:], op=ALU.mult)
                        qro = rop.tile([HD, TPC], F32, tag="qro")
                        nc.vector.tensor_tensor(qro[:], t1[:], t2[:], op=ALU.add)
                        dst = qT_sp if is_q else kT_sp
                        hloc = m % HPC
                        nc.sync.dma_start(dst.ap()[hloc * HD:(hloc + 1) * HD, tb * TPC:(tb + 1) * TPC], qro[:])
                    # v matmuls (natural layout)
                    for ts_ in range(TPC // 128):
                        p_v = ps_mm.tile([128, DPC], F32, tag="pv")
                        for k in range(KH):
                            nc.tensor.matmul(p_v[:], lt[k][:, ts_ * 128:(ts_ + 1) * 128], vw_t[:, k, :], start=(k == 0), stop=(k == KH - 1))
                        vo = rop.tile([128, DPC], F32, tag="vo")
                        nc.vector.tensor_copy(vo[:], p_v[:])
                        nc.sync.dma_start(v_sp.ap()[tb * TPC + ts_ * 128: tb * TPC + (ts_ + 1) * 128, :], vo[:])

            # ---------------- phase B: attention ----------------
            with contextlib.ExitStack() as ph:
                qk_l = ph.enter_context(tc.tile_pool(name="qk_l", bufs=2))
                vt_l = ph.enter_context(tc.tile_pool(name="vt_l", bufs=2))
                pb = ph.enter_context(tc.tile_pool(name="pb", bufs=6))
                ep = ph.enter_context(tc.tile_pool(name="ep", bufs=3))
                ps_sc = ph.enter_context(tc.tile_pool(name="ps_sc", bufs=3, space="PSUM"))
                ps_at = ph.enter_context(tc.tile_pool(name="ps_at", bufs=2, space="PSUM"))
                ps_dn = ph.enter_context(tc.tile_pool(name="ps_dn", bufs=2, space="PSUM"))
                ps_b2 = ph.enter_context(tc.tile_pool(name="ps_b2", bufs=1, space="PSUM"))

                for hloc in range(HPC):
                    for b in range(B):
                        kt_s = qk_l.tile([HD, S], F32, tag="kts")
                        nc.sync.dma_start(kt_s[:], kT_sp.ap()[hloc * HD:(hloc + 1) * HD, b * S:(b + 1) * S])
                        qt_s = qk_l.tile([HD, S], F32, tag="qts")
                        nc.sync.dma_start(qt_s[:], qT_sp.ap()[hloc * HD:(hloc + 1) * HD, b * S:(b + 1) * S])
                        vts = []
                        for kt in range(NKT):
                            vv = vt_l.tile([128, HD], F32, tag=f"v{kt}")
                            nc.sync.dma_start(vv[:], v_sp.ap()[b * S + kt * 128: b * S + (kt + 1) * 128, hloc * HD:(hloc + 1) * HD])
                            vts.append(vv)
                        for qb in range(QPB):
                            kts_used = [kt for kt in range(NKT) if sched[qb][kt] != "skip"]
                            p_at = ps_at.tile([HD, TPC], F32, tag="pat")
                            p_dn = ps_dn.tile([1, TPC], F32, tag="pdn")
                            for i, kt in enumerate(kts_used):
                                st, sp = (i == 0), (i == len(kts_used) - 1)
                                p_sc = ps_sc.tile([128, TPC], F32, tag="psc")
                                nc.tensor.matmul(p_sc[:], kt_s[:, kt * 128:(kt + 1) * 128], qt_s[:, qb * TPC:(qb + 1) * TPC], start=True, stop=True)
                                pr = pb.tile([128, TPC], F32, tag="probs")
                                c = sched[qb][kt]
                                if c != "keep":
                                    mm_ = pb.tile([128, TPC], F32, tag="masked")
                                    nc.vector.tensor_tensor(mm_[:], p_sc[:], mask_t[:, c, :], op=ALU.add)
                                    nc.scalar.activation(pr[:], mm_[:], AF.Exp)
                                else:
                                    nc.scalar.activation(pr[:], p_sc[:], AF.Exp)
                                nc.tensor.matmul(p_at[:], vts[kt][:], pr[:], start=st, stop=sp)
                                nc.tensor.matmul(p_dn[:], ones[:], pr[:], start=st, stop=sp)
                            rec = ep.tile([1, TPC], F32, tag="rec")
                            nc.vector.reciprocal(rec[:], p_dn[:])
                            nc.vector.tensor_scalar(rec[:], rec[:], qscale / oqs, None, op0=ALU.mult)
                            p_b2 = ps_b2.tile([128, TPC], F32, tag="pb2")
                            nc.tensor.matmul(p_b2[:], ones1[:], rec[:], start=True, stop=True)
                            bc = ep.tile([128, TPC], F32, tag="bc")
                            nc.vector.tensor_copy(bc[:], p_b2[:])
                            aqf = ep.tile([HD, TPC], F32, tag="aqf")
                            nc.vector.tensor_tensor(aqf[:], p_at[:], bc[:], op=ALU.mult)
                            nc.vector.tensor_scalar(aqf[:], aqf[:], MAGIC, MAGIC, op0=ALU.add, op1=ALU.subtract)
                            aqb = ep.tile([HD, TPC], BF16, tag="aqb")
                            nc.vector.tensor_scalar(aqb[:], aqf[:], 127.0, -127.0, op0=ALU.min, op1=ALU.max)
                            nc.sync.dma_start(aq_out.ap()[hloc * HD:(hloc + 1) * HD, b * S + qb * TPC: b * S + (qb + 1) * TPC], aqb[:])
    nc.compile()
    return nc


def build_prog2(H, I, TPC, o_scale, gate_scale, upds, down_scale):
    """o-proj + ln2 + MLP for TPC tokens per core."""
    KH = H // 128
    KI = I // 128
    nc = bacc.Bacc("TRN2", target_bir_lowering=False, debug=False, num_devices=NC_)
    aq = nc.dram_tensor("aq_T", [H, TPC], BF16, kind="ExternalInput")
    resid = nc.dram_tensor("resid_T", [H, TPC], F32, kind="ExternalInput")
    o_w = nc.dram_tensor("o_wT", [H, H], BF16, kind="ExternalInput")
    gate_w = nc.dram_tensor("gate_wT", [H, I], BF16, kind="ExternalInput")
    up_w = nc.dram_tensor("up_wT", [H, I], BF16, kind="ExternalInput")
    down_w = nc.dram_tensor("down_wT", [I, H], BF16, kind="ExternalInput")
    rms2 = nc.dram_tensor("rms2", [128, KH], F32, kind="ExternalInput")
    out_T = nc.dram_tensor("out_T", [H, TPC], F32, kind="ExternalOutput")
    h_sp = nc.dram_tensor("h_sp", [H, TPC], F32)

    with tile.TileContext(nc) as tc:
        import contextlib
        with contextlib.ExitStack() as top:
            const = top.enter_context(tc.tile_pool(name="const", bufs=1))
            ones = const.tile([128, 1], F32, tag="ones")
            nc.vector.memset(ones[:], 1.0)
            ones1 = const.tile([1, 128], F32, tag="ones1")
            nc.vector.memset(ones1[:], 1.0)
            rms2_t = const.tile([128, KH], F32, tag="rms2t")
            nc.sync.dma_start(rms2_t[:], rms2.ap())
            ln2q_p = top.enter_context(tc.tile_pool(name="ln2q", bufs=1))
            mq_p = top.enter_context(tc.tile_pool(name="mq", bufs=1))

            ln2q = [ln2q_p.tile([128, TPC], BF16, tag=f"l2q{k}", name=f"l2q{k}") for k in range(KH)]
            mq = [mq_p.tile([128, TPC], BF16, tag=f"mq{k}", name=f"mq{k}") for k in range(KI)]

            # ---- o-proj + h + ln2 ----
            with contextlib.ExitStack() as ph:
                aq_l = ph.enter_context(tc.tile_pool(name="aq_l", bufs=1))
                wst = ph.enter_context(tc.tile_pool(name="wst", bufs=6))
                hb = ph.enter_context(tc.tile_pool(name="hb", bufs=2))
                sm = ph.enter_context(tc.tile_pool(name="sm", bufs=1))
                ps_o = ph.enter_context(tc.tile_pool(name="ps_o", bufs=4, space="PSUM"))
                ps_ss = ph.enter_context(tc.tile_pool(name="ps_ss", bufs=1, space="PSUM"))
                ps_bc = ph.enter_context(tc.tile_pool(name="ps_bc", bufs=1, space="PSUM"))

                aqt = []
                for k in range(KH):
                    a = aq_l.tile([128, TPC], BF16, tag=f"aq{k}")
                    nc.sync.dma_start(a[:], aq.ap()[k * 128:(k + 1) * 128, :])
                    aqt.append(a)
                p_ss = ps_ss.tile([1, TPC], F32, tag="pss")
                hts = []
                for m in range(KH):
                    p_o = ps_o.tile([128, TPC], F32, tag="po")
                    for k in range(KH):
                        wt = wst.tile([128, 128], BF16, tag="wo")
                        nc.sync.dma_start(wt[:], o_w.ap()[k * 128:(k + 1) * 128, m * 128:(m + 1) * 128])
                        nc.tensor.matmul(p_o[:], wt[:], aqt[k][:], start=(k == 0), stop=(k == KH - 1))
                    rs = hb.tile([128, TPC], F32, tag="rs")
                    nc.sync.dma_start(rs[:], resid.ap()[m * 128:(m + 1) * 128, :])
                    ht = hb.tile([128, TPC], F32, tag=f"ht{m % 4}")
                    nc.vector.scalar_tensor_tensor(ht[:], p_o[:], o_scale, rs[:], op0=ALU.mult, op1=ALU.add)
                    nc.sync.dma_start(h_sp.ap()[m * 128:(m + 1) * 128, :], ht[:])
                    z2 = hb.tile([128, TPC], F32, tag="z2")
                    nc.scalar.activation(z2[:], ht[:], AF.Square)
                    nc.tensor.matmul(p_ss[:], ones[:], z2[:], start=(m == 0), stop=(m == KH - 1))
                    hts.append(ht)
                vt = sm.tile([1, TPC], F32, tag="vt")
                nc.vector.tensor_scalar(vt[:], p_ss[:], 1.0 / H, EPS, op0=ALU.mult, op1=ALU.add)
                rt = sm.tile([1, TPC], F32, tag="rt")
                nc.vector.reciprocal(rt[:], vt[:])
                yt = sm.tile([1, TPC], F32, tag="yt")
                nc.scalar.activation(yt[:], rt[:], AF.Sqrt)
                for it_ in range(2):
                    u = sm.tile([1, TPC], F32, tag=f"u_nw{it_}", name=f"u_nw{it_}")
                    nc.vector.tensor_tensor(u[:], yt[:], yt[:], op=ALU.mult)
                    nc.vector.tensor_tensor(u[:], u[:], vt[:], op=ALU.mult)
                    nc.vector.tensor_scalar(u[:], u[:], -0.5, 1.5, op0=ALU.mult, op1=ALU.add)
                    y2 = sm.tile([1, TPC], F32, tag=f"y2_nw{it_}", name=f"y2_nw{it_}")
                    nc.vector.tensor_tensor(y2[:], yt[:], u[:], op=ALU.mult)
                    yt = y2
                p_bc = ps_bc.tile([128, TPC], F32, tag="pbc")
                nc.tensor.matmul(p_bc[:], ones1[:], yt[:], start=True, stop=True)
                rs_bc = sm.tile([128, TPC], F32, tag="rsbc")
                nc.vector.tensor_copy(rs_bc[:], p_bc[:])
                for m in range(KH):
                    hh = hb.tile([128, TPC], F32, tag="hh")
                    nc.sync.dma_start(hh[:], h_sp.ap()[m * 128:(m + 1) * 128, :])
                    f = hb.tile([128, TPC], F32, tag="f2")
                    nc.vector.scalar_tensor_tensor(f[:], hh[:], rms2_t[:, m:m + 1], rs_bc[:], op0=ALU.mult, op1=ALU.mult)
                    nc.vector.tensor_scalar(f[:], f[:], MAGIC, MAGIC, op0=ALU.add, op1=ALU.subtract)
                    nc.vector.tensor_scalar(ln2q[m][:], f[:], 127.0, -127.0, op0=ALU.min, op1=ALU.max)

            # ---- gate/up ----
            with contextlib.ExitStack() as ph:
                wst = ph.enter_context(tc.tile_pool(name="wst2", bufs=6))
                eb = ph.enter_context(tc.tile_pool(name="eb", bufs=4))
                ps_g = ph.enter_context(tc.tile_pool(name="ps_g", bufs=1, space="PSUM"))
                ps_u = ph.enter_context(tc.tile_pool(name="ps_u", bufs=1, space="PSUM"))
                GRP = 4
                for ig in range(_ceil_div(KI, GRP)):
                    i0 = ig * GRP
                    nI = min(GRP, KI - i0)
                    pgs = [ps_g.tile([128, TPC], F32, tag=f"pg{j}", name=f"pg{j}") for j in range(nI)]
                    pus = [ps_u.tile([128, TPC], F32, tag=f"pu{j}", name=f"pu{j}") for j in range(nI)]
                    for k in range(KH):
                        gw = wst.tile([128, nI * 128], BF16, tag="gw")
                        nc.sync.dma_start(gw[:], gate_w.ap()[k * 128:(k + 1) * 128, i0 * 128:(i0 + nI) * 128])
                        uw = wst.tile([128, nI * 128], BF16, tag="uw")
                        nc.sync.dma_start(uw[:], up_w.ap()[k * 128:(k + 1) * 128, i0 * 128:(i0 + nI) * 128])
                        for j in range(nI):
                            nc.tensor.matmul(pgs[j][:], gw[:, j * 128:(j + 1) * 128], ln2q[k][:], start=(k == 0), stop=(k == KH - 1))
                            nc.tensor.matmul(pus[j][:], uw[:, j * 128:(j + 1) * 128], ln2q[k][:], start=(k == 0), stop=(k == KH - 1))
                    for j in range(nI):
                        sg = eb.tile([128, TPC], F32, tag="sg")
                        nc.scalar.activation(sg[:], pgs[j][:], AF.Sigmoid, scale=gate_scale)
                        gt = eb.tile([128, TPC], F32, tag="gt")
                        nc.scalar.mul(gt[:], pgs[j][:], gate_scale)
                        f = eb.tile([128, TPC], F32, tag="f3")
                        nc.vector.scalar_tensor_tensor(f[:], pus[j][:], upds, sg[:], op0=ALU.mult, op1=ALU.mult)
                        nc.vector.tensor_tensor(f[:], f[:], gt[:], op=ALU.mult)
                        nc.vector.tensor_scalar(f[:], f[:], MAGIC, MAGIC, op0=ALU.add, op1=ALU.subtract)
                        nc.vector.tensor_scalar(mq[i0 + j][:], f[:], 127.0, -127.0, op0=ALU.min, op1=ALU.max)

            # ---- down + final ----
            with contextlib.ExitStack() as ph:
                wst = ph.enter_context(tc.tile_pool(name="wst3", bufs=6))
                eb = ph.enter_context(tc.tile_pool(name="eb2", bufs=4))
                ps_d = ph.enter_context(tc.tile_pool(name="ps_d", bufs=1, space="PSUM"))
                GRP = 4
                for mg in range(_ceil_div(KH, GRP)):
                    m0 = mg * GRP
                    nM = min(GRP, KH - m0)
                    pds = [ps_d.tile([128, TPC], F32, tag=f"pd{j}", name=f"pd{j}") for j in range(nM)]
                    for k in range(KI):
                        dw = wst.tile([128, nM * 128], BF16, tag="dw")
                        nc.sync.dma_start(dw[:], down_w.ap()[k * 128:(k + 1) * 128, m0 * 128:(m0 + nM) * 128])
                        for j in range(nM):
                            nc.tensor.matmul(pds[j][:], dw[:, j * 128:(j + 1) * 128], mq[k][:], start=(k == 0), stop=(k == KI - 1))
                    for j in range(nM):
                        m = m0 + j
                        hh = eb.tile([128, TPC], F32, tag="hh2")
                        nc.sync.dma_start(hh[:], h_sp.ap()[m * 128:(m + 1) * 128, :])
                        ot = eb.tile([128, TPC], F32, tag="ot")
                        nc.vector.scalar_tensor_tensor(ot[:], pds[j][:], down_scale, hh[:], op0=ALU.mult, op1=ALU.add)
                        nc.sync.dma_start(out_T.ap()[m * 128:(m + 1) * 128, :], ot[:])
    nc.compile()
    return nc


def _mask_schedule(mask, S, TPC):
    """Classify [qb, kt] tiles of mask^T: 'keep' (all 0), 'skip' (all <=-1e8),
    else index into deduped partial-tile array."""
    NKT = S // 128
    QPB = S // TPC
    mT = np.ascontiguousarray(mask.T)  # [kt, qt]
    sched = [[None] * NKT for _ in range(QPB)]
    tiles = []
    keys = {}
    for qb in range(QPB):
        for kt in range(NKT):
            sub = mT[kt * 128:(kt + 1) * 128, qb * TPC:(qb + 1) * TPC]
            if np.all(sub == 0):
                sched[qb][kt] = "keep"
            elif np.all(sub <= -1e8):
                sched[qb][kt] = "skip"
            else:
                key = sub.tobytes()
                if key not in keys:
                    keys[key] = len(tiles)
                    tiles.append(sub.astype(np.float32))
                sched[qb][kt] = keys[key]
    if not tiles:
        tiles = [np.zeros((128, TPC), np.float32)]
    return sched, np.stack(tiles)


def _rope_tables(S, HD, qkv_scale):
    inv = 1.0 / (10000.0 ** (np.arange(0, HD, 2, dtype=np.float32) / HD))
    freqs = np.outer(np.arange(S, dtype=np.float32), inv)
    emb = np.concatenate([freqs, freqs], axis=-1)  # [S, HD]
    cos = np.cos(emb).T.astype(np.float64)  # [HD, S]
    sin = np.sin(emb).T.astype(np.float64)
    hh = HD // 2
    sgn = np.ones((HD, 1))
    sgn[:hh] = -1.0
    # sin table is pre-swapped+signed: row d holds sign(d)*sin[sigma(d)] where
    # sigma swaps halves -- because the kernel multiplies the SHIFTED q by it.
    sin_sw = np.concatenate([sin[hh:], sin[:hh]], axis=0)
    sq = math.sqrt(HD)
    cosq = (cos * qkv_scale / sq).astype(np.float32)
    sinq = (sin_sw * sgn * qkv_scale / sq).astype(np.float32)
    cosk = (cos * qkv_scale).astype(np.float32)
    sink = (sin_sw * sgn * qkv_scale).astype(np.float32)
    return cosq, sinq, cosk, sink


def kernel(hidden_states, attention_mask, rms1_w, rms2_w, qkv_w, o_w, gate_w,
           up_w, down_w, qkv_scale, o_quant_scale, o_scale, gate_scale,
           up_scale, down_quant_scale, down_scale):
    B, S, H = hidden_states.shape
    NH, HD = 32, 128
    I = gate_w.shape[0]
    T = B * S
    TPC = T // NC_
    KH = H // 128
    HPC = NH // NC_
    DPC = HPC * HD
    bf = ml_dtypes.bfloat16

    hid_T = np.ascontiguousarray(hidden_states.reshape(T, H).T.astype(np.float32))
    sched, maskp = _mask_schedule(np.asarray(attention_mask)[0, 0], S, TPC)
    cosq, sinq, cosk, sink = _rope_tables(S, HD, float(qkv_scale))
    rms1_t = np.ascontiguousarray(np.asarray(rms1_w, np.float32).reshape(KH, 128).T)
    rms2_t = np.ascontiguousarray(np.asarray(rms2_w, np.float32).reshape(KH, 128).T)

    qkv_w = np.asarray(qkv_w)
    prog1 = build_prog1(B, S, H, NH, HD, TPC, sched, maskp.shape[0],
                        float(qkv_scale), float(o_quant_scale))
    in1 = []
    for c in range(NC_):
        qs = qkv_w[c * DPC:(c + 1) * DPC]              # q rows
        ks = qkv_w[H + c * DPC: H + (c + 1) * DPC]     # k rows
        vs = qkv_w[2 * H + c * DPC: 2 * H + (c + 1) * DPC]
        qk_wT = np.ascontiguousarray(np.concatenate([qs, ks], 0).T).astype(bf)
        v_wT = np.ascontiguousarray(vs.T).astype(bf)
        in1.append({
            "hid_T": hid_T, "qk_wT": qk_wT, "v_wT": v_wT,
            "cosq": cosq, "sinq": sinq, "cosk": cosk, "sink": sink,
            "rms1": rms1_t, "maskp": maskp,
        })
    trace = bool(os.environ.get("KTRACE"))
    global LAST_EXEC_NS
    LAST_EXEC_NS = []
    res1 = run_bass_kernel_spmd(prog1, in1, core_ids=list(range(NC_)), trace=trace)
    if res1.exec_time_ns:
        LAST_EXEC_NS.append(res1.exec_time_ns)
    aq_full = np.concatenate([r["aq_out"] for r in res1.results], axis=0)  # [H, T]

    prog2 = build_prog2(H, I, TPC, float(o_scale), float(gate_scale),
                        float(up_scale) / float(down_quant_scale), float(down_scale))
    o_wT = np.ascontiguousarray(np.asarray(o_w).T).astype(bf)
    gate_wT = np.ascontiguousarray(np.asarray(gate_w).T).astype(bf)
    up_wT = np.ascontiguousarray(np.asarray(up_w).T).astype(bf)
    down_wT = np.ascontiguousarray(np.asarray(down_w).T).astype(bf)
    in2 = []
    for c in range(NC_):
        tok = slice(c * TPC, (c + 1) * TPC)
        in2.append({
            "aq_T": np.ascontiguousarray(aq_full[:, tok]),
            "resid_T": np.ascontiguousarray(hid_T[:, tok]),
            "o_wT": o_wT, "gate_wT": gate_wT, "up_wT": up_wT, "down_wT": down_wT,
            "rms2": rms2_t,
        })
    res2 = run_bass_kernel_spmd(prog2, in2, core_ids=list(range(NC_)), trace=trace)
    if res2.exec_time_ns:
        LAST_EXEC_NS.append(res2.exec_time_ns)
    out_T = np.concatenate([r["out_T"] for r in res2.results], axis=1)  # [H, T]
    return np.ascontiguousarray(out_T.T).reshape(B, S, H).astype(np.float32)



# revision 34
# speedup vs baseline: 27548.8102x; 27548.8102x over previous
"""Int8 Baichuan layer on 8 trn2 cores.

Design:
  Program 1 (heads-TP): every core normalizes+quantizes all tokens (ln1),
    computes q/k/v for ITS 4 heads (weights sharded on output dim), RoPE,
    causal attention (scores fp32, probs/v fp16), quantizes attn output
    -> aq [512 dims, 4096 tok] fp16.
  Host: redistributes aq from head-sharding to token-sharding.
  Program 2 (token-split): each core takes 512 tokens with full attn dims:
    o-proj + residual, ln2, gate/up/silu/mul/quant, down-proj + residual.
  Host: concatenates per-core [H, 512] outputs, transposes to [B,S,H].

All "int8" matmuls run as fp16 (exact for |int|<=127, fp32 PSUM accum).
Score matmuls run fp32 (precision-critical: quant boundaries downstream).
Quantization = magic-number rint (fp32) + clamp on DVE.
"""
import sys
sys.path.insert(0, "/opt/trn_rl_repo")
import math
import os
import numpy as np
import ml_dtypes

import concourse.bass as bass
import concourse.bacc as bacc
import concourse.tile as tile
from concourse import mybir
from concourse.bass_utils import run_bass_kernel_spmd

F32 = mybir.dt.float32
F16 = mybir.dt.float16
AF = mybir.ActivationFunctionType
ALU = mybir.AluOpType
MAGIC = float(np.float32(1.5 * 2 ** 23))
NC_ = 8
EPS = 1e-6


def _ceil_div(a, b):
    return (a + b - 1) // b


def _newton_rsqrt(nc, sm, vt, TPC):
    """rsqrt(vt) [1,TPC] via reciprocal+sqrt+2 Newton steps (ACT Rsqrt is banned)."""
    rt = sm.tile([1, TPC], F32, tag="rt")
    nc.vector.reciprocal(rt[:], vt[:])
    yt = sm.tile([1, TPC], F32, tag="yt")
    nc.scalar.activation(yt[:], rt[:], AF.Sqrt)
    for it_ in range(2):
        u = sm.tile([1, TPC], F32, tag=f"u_nw{it_}", name=f"u_nw{it_}")
        nc.vector.tensor_tensor(u[:], yt[:], yt[:], op=ALU.mult)
        nc.vector.tensor_tensor(u[:], u[:], vt[:], op=ALU.mult)
        nc.vector.tensor_scalar(u[:], u[:], -0.5, 1.5, op0=ALU.mult, op1=ALU.add)
        y2 = sm.tile([1, TPC], F32, tag=f"y2_nw{it_}", name=f"y2_nw{it_}")
        nc.vector.tensor_tensor(y2[:], yt[:], u[:], op=ALU.mult)
        yt = y2
    return yt


def build_prog1(B, S, H, NH, HD, TPC, sched, n_maskp, qscale, oqs):
    """ln1 + qkv + rope + attention + quant. Per-core SPMD program."""
    T = B * S
    KH = H // 128          # h k-tiles
    HPC = NH // NC_        # heads per core
    DPC = HPC * HD         # q dims per core
    TB = T // TPC          # token blocks
    QPB = S // TPC         # q blocks per batch
    NKT = S // 128         # kt tiles per batch

    nc = bacc.Bacc("TRN2", target_bir_lowering=False, debug=False, num_devices=NC_)
    hid = nc.dram_tensor("hid_T", [H, T], F32, kind="ExternalInput")
    qk_w = nc.dram_tensor("qk_wT", [H, 2 * DPC], F16, kind="ExternalInput")
    v_w = nc.dram_tensor("v_wT", [H, DPC], F16, kind="ExternalInput")
    cosq = nc.dram_tensor("cosq", [HD, S], F32, kind="ExternalInput")
    sinq = nc.dram_tensor("sinq", [HD, S], F32, kind="ExternalInput")
    cosk = nc.dram_tensor("cosk", [HD, S], F32, kind="ExternalInput")
    sink = nc.dram_tensor("sink", [HD, S], F32, kind="ExternalInput")
    rms1 = nc.dram_tensor("rms1", [128, KH], F32, kind="ExternalInput")
    maskp = nc.dram_tensor("maskp", [n_maskp, 128, TPC], F32, kind="ExternalInput")
    aq_out = nc.dram_tensor("aq_out", [DPC, T], F16, kind="ExternalOutput")

    qT_sp = nc.dram_tensor("qT_sp", [DPC, T], F32)
    kT_sp = nc.dram_tensor("kT_sp", [DPC, T], F32)
    v_sp = nc.dram_tensor("v_sp", [T, DPC], F32)

    with tile.TileContext(nc) as tc:
        import contextlib
        with contextlib.ExitStack() as top:
            const = top.enter_context(tc.tile_pool(name="const", bufs=1))
            onesf = const.tile([128, 1], F32, tag="onesf")
            nc.vector.memset(onesf[:], 1.0)
            ones1f = const.tile([1, 128], F32, tag="ones1f")
            nc.vector.memset(ones1f[:], 1.0)
            rms1_t = const.tile([128, KH], F32, tag="rms1t")
            nc.sync.dma_start(rms1_t[:], rms1.ap())
            mask_t = const.tile([128, n_maskp, TPC], F32, tag="maskt")
            for i in range(n_maskp):
                nc.sync.dma_start(mask_t[:, i, :], maskp.ap()[i, :, :])
            vw_t = const.tile([128, KH, DPC], F16, tag="vwt")
            for k in range(KH):
                nc.sync.dma_start(vw_t[:, k, :], v_w.ap()[k * 128:(k + 1) * 128, :])

            # ---------------- phase A: ln1 + qkv + rope ----------------
            with contextlib.ExitStack() as ph:
                xsq = ph.enter_context(tc.tile_pool(name="xsq", bufs=2))
                xln = ph.enter_context(tc.tile_pool(name="xln", bufs=2))
                lnq = ph.enter_context(tc.tile_pool(name="lnq", bufs=2))
                wst = ph.enter_context(tc.tile_pool(name="wst", bufs=4))
                rop = ph.enter_context(tc.tile_pool(name="rop", bufs=2))
                sm = ph.enter_context(tc.tile_pool(name="sm", bufs=1))
                ps_ss = ph.enter_context(tc.tile_pool(name="ps_ss", bufs=1, space="PSUM"))
                ps_bc = ph.enter_context(tc.tile_pool(name="ps_bc", bufs=1, space="PSUM"))
                ps_mm = ph.enter_context(tc.tile_pool(name="ps_mm", bufs=2, space="PSUM"))

                hidR = hid.ap().rearrange("(kh p) t -> p kh t", p=128)
                qkR = qk_w.ap().rearrange("(kh p) m -> p kh m", p=128)
                KB = KH // 4  # 4 k-tiles per DMA
                for tb in range(TB):
                    b = tb // QPB
                    s0 = (tb % QPB) * TPC
                    # sum of squares over H via ones-matmul on x^2
                    p_ss = ps_ss.tile([1, TPC], F32, tag="pss")
                    for kb in range(KH // 2):
                        xt2 = xsq.tile([128, 2, TPC], F32, tag="xt2")
                        nc.sync.dma_start(xt2[:], hidR[:, kb * 2:(kb + 1) * 2, tb * TPC:(tb + 1) * TPC])
                        zt2 = xsq.tile([128, 2, TPC], F32, tag="zt2")
                        nc.scalar.activation(zt2[:], xt2[:], AF.Square)
                        for kk in range(2):
                            k = kb * 2 + kk
                            nc.tensor.matmul(p_ss[:], onesf[:], zt2[:, kk, :], start=(k == 0), stop=(k == KH - 1))
                    # rsqrt(mean + eps)
                    vt = sm.tile([1, TPC], F32, tag="vt")
                    nc.vector.tensor_scalar(vt[:], p_ss[:], 1.0 / H, EPS, op0=ALU.mult, op1=ALU.add)
                    yt = _newton_rsqrt(nc, sm, vt, TPC)
                    p_bc = ps_bc.tile([128, TPC], F32, tag="pbc")
                    nc.tensor.matmul(p_bc[:], ones1f[:], yt[:], start=True, stop=True)
                    rs_bc = sm.tile([128, TPC], F32, tag="rsbc")
                    nc.vector.tensor_copy(rs_bc[:], p_bc[:])
                    # ln1q tiles (fp16 ints)
                    lt = []
                    for kb in range(KB):
                        x2_4 = xln.tile([128, 4, TPC], F32, tag="x2_4")
                        nc.sync.dma_start(x2_4[:], hidR[:, kb * 4:(kb + 1) * 4, tb * TPC:(tb + 1) * TPC])
                        for kk in range(4):
                            k = kb * 4 + kk
                            f = xln.tile([128, TPC], F32, tag="f")
                            nc.vector.scalar_tensor_tensor(f[:], x2_4[:, kk, :], rms1_t[:, k:k + 1], rs_bc[:], op0=ALU.mult, op1=ALU.mult)
                            q8 = lnq.tile([128, TPC], F16, tag=f"lnq{k}")
                            nc.vector.tensor_scalar(q8[:], f[:], MAGIC, MAGIC, op0=ALU.add, op1=ALU.subtract)
                            lt.append(q8)
                    # q/k matmuls + rope
                    cq = rop.tile([HD, TPC], F32, tag="cq")
                    nc.sync.dma_start(cq[:], cosq.ap()[:, s0:s0 + TPC])
                    sq = rop.tile([HD, TPC], F32, tag="sq")
                    nc.sync.dma_start(sq[:], sinq.ap()[:, s0:s0 + TPC])
                    ck = rop.tile([HD, TPC], F32, tag="ck")
                    nc.sync.dma_start(ck[:], cosk.ap()[:, s0:s0 + TPC])
                    sk = rop.tile([HD, TPC], F32, tag="sk")
                    nc.sync.dma_start(sk[:], sink.ap()[:, s0:s0 + TPC])
                    for mp in range(HPC):  # pairs of m-tiles; 4 k-tiles per weight DMA
                        p_qk2 = [ps_mm.tile([128, TPC], F32, tag=f"pqk{j}", name=f"pqk{j}") for j in range(2)]
                        for kb in range(KB):
                            wt4 = wst.tile([128, 4, 256], F16, tag="wqk4")
                            nc.sync.dma_start(wt4[:], qkR[:, kb * 4:(kb + 1) * 4, mp * 256:(mp + 1) * 256])
                            for kk in range(4):
                                k = kb * 4 + kk
                                for j in range(2):
                                    nc.tensor.matmul(p_qk2[j][:], wt4[:, kk, j * 128:(j + 1) * 128], lt[k][:], start=(k == 0), stop=(k == KH - 1))
                        for j in range(2):
                            m = mp * 2 + j
                            is_q = m < HPC
                            p_qk = p_qk2[j]
                            ctab = cq if is_q else ck
                            stab = sq if is_q else sk
                            t1 = rop.tile([HD, TPC], F32, tag="t1")
                            nc.vector.tensor_tensor(t1[:], p_qk[:], ctab[:], op=ALU.mult)
                            qs = rop.tile([HD, TPC], F32, tag="qs")
                            nc.vector.tensor_copy(qs[:], p_qk[:])
                            qsh = rop.tile([HD, TPC], F32, tag="qsh")
                            hh = HD // 2
                            nc.sync.dma_start(qsh[0:hh, :], qs[hh:HD, :])
                            nc.sync.dma_start(qsh[hh:HD, :], qs[0:hh, :])
                            t2 = rop.tile([HD, TPC], F32, tag="t2")
                            nc.vector.tensor_tensor(t2[:], qsh[:], stab[:], op=ALU.mult)
                            qro = rop.tile([HD, TPC], F32, tag="qro")
                            nc.vector.tensor_tensor(qro[:], t1[:], t2[:], op=ALU.add)
                            dst = qT_sp if is_q else kT_sp
                            hloc = m % HPC
                            nc.sync.dma_start(dst.ap()[hloc * HD:(hloc + 1) * HD, tb * TPC:(tb + 1) * TPC], qro[:])
                    # v matmuls (natural layout); scale by qkv_scale -> fp16
                    for ts_ in range(TPC // 128):
                        p_v = ps_mm.tile([128, DPC], F32, tag=f"pqk{ts_ % 2}", name=f"pv{ts_}")
                        for k in range(KH):
                            nc.tensor.matmul(p_v[:], lt[k][:, ts_ * 128:(ts_ + 1) * 128], vw_t[:, k, :], start=(k == 0), stop=(k == KH - 1))
                        vo = rop.tile([128, DPC], F32, tag="vo")
                        nc.vector.tensor_scalar(vo[:], p_v[:], qscale, None, op0=ALU.mult)
                        nc.sync.dma_start(v_sp.ap()[tb * TPC + ts_ * 128: tb * TPC + (ts_ + 1) * 128, :], vo[:])

            # ---------------- phase B: attention ----------------
            # scores fp32 (moving q fp32); probs fp16 (exp output), v fp16.
            with contextlib.ExitStack() as ph:
                qk_l = ph.enter_context(tc.tile_pool(name="qk_l", bufs=2))
                vt_l = ph.enter_context(tc.tile_pool(name="vt_l", bufs=2))
                pb = ph.enter_context(tc.tile_pool(name="pb", bufs=4))
                ep = ph.enter_context(tc.tile_pool(name="ep", bufs=3))
                ps_sc = ph.enter_context(tc.tile_pool(name="ps_sc", bufs=2, space="PSUM"))
                ps_at = ph.enter_context(tc.tile_pool(name="ps_at", bufs=2, space="PSUM"))
                ps_dn = ph.enter_context(tc.tile_pool(name="ps_dn", bufs=1, space="PSUM"))
                ps_b2 = ph.enter_context(tc.tile_pool(name="ps_b2", bufs=1, space="PSUM"))

                v_spR = v_sp.ap().rearrange("(nt p) d -> p nt d", p=128)
                for hloc in range(HPC):
                    for b in range(B):
                        kt_s = qk_l.tile([HD, S], F32, tag="kts")
                        nc.sync.dma_start(kt_s[:], kT_sp.ap()[hloc * HD:(hloc + 1) * HD, b * S:(b + 1) * S])
                        qt_s = qk_l.tile([HD, S], F32, tag="qts")
                        nc.sync.dma_start(qt_s[:], qT_sp.ap()[hloc * HD:(hloc + 1) * HD, b * S:(b + 1) * S])
                        vts = []
                        for ktb in range(NKT // 4):
                            vv4 = vt_l.tile([128, 4, HD], F32, tag=f"v{ktb}")
                            nt0 = b * NKT + ktb * 4
                            nc.sync.dma_start(vv4[:], v_spR[:, nt0:nt0 + 4, hloc * HD:(hloc + 1) * HD])
                            for i in range(4):
                                vts.append(vv4[:, i, :])
                        for qb in range(QPB):
                            kts_used = [kt for kt in range(NKT) if sched[qb][kt] != "skip"]
                            pairs = [kts_used[i:i + 2] for i in range(0, len(kts_used), 2)]
                            p_at = ps_at.tile([HD, TPC], F32, tag="pat")
                            p_dn = ps_dn.tile([1, TPC], F32, tag="pdn")
                            first = True
                            for pi, pair in enumerate(pairs):
                                np_ = len(pair)
                                p_sc = ps_sc.tile([128, 2, TPC], F32, tag="psc")
                                for i, kt in enumerate(pair):
                                    nc.tensor.matmul(p_sc[:, i, :], kt_s[:, kt * 128:(kt + 1) * 128], qt_s[:, qb * TPC:(qb + 1) * TPC], start=True, stop=True)
                                pr = pb.tile([128, 2, TPC], F32, tag="probs")
                                if np_ == 2:
                                    nc.scalar.activation(pr[:, 0:2, :], p_sc[:, 0:2, :], AF.Exp)
                                else:
                                    nc.scalar.activation(pr[:, 0, :], p_sc[:, 0, :], AF.Exp)
                                for i, kt in enumerate(pair):
                                    c = sched[qb][kt]
                                    if c != "keep":
                                        nc.vector.tensor_tensor(pr[:, i, :], pr[:, i, :], mask_t[:, c, :], op=ALU.mult)
                                for i, kt in enumerate(pair):
                                    last = (pi == len(pairs) - 1) and (i == np_ - 1)
                                    nc.tensor.matmul(p_at[:], vts[kt], pr[:, i, :], start=first, stop=last)
                                    nc.tensor.matmul(p_dn[:], onesf[:], pr[:, i, :], start=first, stop=last)
                                    first = False
                            rec = ep.tile([1, TPC], F32, tag="rec")
                            scr = ep.tile([1, TPC], F32, tag="scr")
                            nc.vector.reciprocal_approx_accurate(out=rec[:], in_=p_dn[:], scratch=scr[:])
                            p_b2 = ps_b2.tile([128, TPC], F32, tag="pb2")
                            nc.tensor.matmul(p_b2[:], ones1f[:], rec[:], start=True, stop=True)
                            bc = ep.tile([128, TPC], F32, tag="bc")
                            nc.vector.tensor_copy(bc[:], p_b2[:])
                            aqf = ep.tile([HD, TPC], F32, tag="aqf")
                            nc.vector.scalar_tensor_tensor(aqf[:], p_at[:], 1.0 / oqs, bc[:], op0=ALU.mult, op1=ALU.mult)
                            nc.vector.tensor_scalar(aqf[:], aqf[:], MAGIC, MAGIC, op0=ALU.add, op1=ALU.subtract)
                            aqb = ep.tile([HD, TPC], F16, tag="aqb")
                            nc.vector.tensor_scalar(aqb[:], aqf[:], 127.0, -127.0, op0=ALU.min, op1=ALU.max)
                            nc.sync.dma_start(aq_out.ap()[hloc * HD:(hloc + 1) * HD, b * S + qb * TPC: b * S + (qb + 1) * TPC], aqb[:])
    nc.compile()
    return nc


def build_prog2(H, I, TPC, o_scale, gate_scale, upds, down_scale):
    """o-proj + ln2 + MLP for TPC tokens per core."""
    KH = H // 128
    KI = I // 128
    nc = bacc.Bacc("TRN2", target_bir_lowering=False, debug=False, num_devices=NC_)
    aq = nc.dram_tensor("aq_T", [H, TPC], F16, kind="ExternalInput")
    resid = nc.dram_tensor("resid_T", [H, TPC], F32, kind="ExternalInput")
    o_w = nc.dram_tensor("o_wT", [H, H], F16, kind="ExternalInput")
    gu_w = nc.dram_tensor("gu_wT", [H, 2 * I], F16, kind="ExternalInput")  # interleaved gate/up, 256-col blocks
    down_w = nc.dram_tensor("down_wT", [I, H], F16, kind="ExternalInput")
    rms2 = nc.dram_tensor("rms2", [128, KH], F32, kind="ExternalInput")
    out_T = nc.dram_tensor("out_T", [H, TPC], F32, kind="ExternalOutput")
    h_sp = nc.dram_tensor("h_sp", [H, TPC], F32)

    with tile.TileContext(nc) as tc:
        import contextlib
        with contextlib.ExitStack() as top:
            const = top.enter_context(tc.tile_pool(name="const", bufs=1))
            ones = const.tile([128, 1], F32, tag="ones")
            nc.vector.memset(ones[:], 1.0)
            ones1 = const.tile([1, 128], F32, tag="ones1")
            nc.vector.memset(ones1[:], 1.0)
            rms2_t = const.tile([128, KH], F32, tag="rms2t")
            nc.sync.dma_start(rms2_t[:], rms2.ap())
            ln2q_p = top.enter_context(tc.tile_pool(name="ln2q", bufs=1))
            mq_p = top.enter_context(tc.tile_pool(name="mq", bufs=1))

            ln2q = [ln2q_p.tile([128, TPC], F16, tag=f"l2q{k}", name=f"l2q{k}") for k in range(KH)]
            mq = [mq_p.tile([128, TPC], F16, tag=f"mq{k}", name=f"mq{k}") for k in range(KI)]

            # ---- o-proj + h + ln2 ----
            with contextlib.ExitStack() as ph:
                aq_l = ph.enter_context(tc.tile_pool(name="aq_l", bufs=1))
                wst = ph.enter_context(tc.tile_pool(name="wst", bufs=6))
                hb = ph.enter_context(tc.tile_pool(name="hb", bufs=2))
                sm = ph.enter_context(tc.tile_pool(name="sm", bufs=1))
                ps_o = ph.enter_context(tc.tile_pool(name="ps_o", bufs=2, space="PSUM"))
                ps_ss = ph.enter_context(tc.tile_pool(name="ps_ss", bufs=1, space="PSUM"))
                ps_bc = ph.enter_context(tc.tile_pool(name="ps_bc", bufs=1, space="PSUM"))

                aqt = []
                for k in range(KH):
                    a = aq_l.tile([128, TPC], F16, tag=f"aq{k}")
                    nc.sync.dma_start(a[:], aq.ap()[k * 128:(k + 1) * 128, :])
                    aqt.append(a)
                p_ss = ps_ss.tile([1, TPC], F32, tag="pss")
                oR = o_w.ap().rearrange("(kh p) m -> p kh m", p=128)
                for mp in range(KH // 2):  # pairs of m-tiles; 4 k-tiles per weight DMA
                    p_o2 = [ps_o.tile([128, TPC], F32, tag=f"po{j}", name=f"po{j}") for j in range(2)]
                    for kb in range(KH // 4):
                        wt4 = wst.tile([128, 4, 256], F16, tag="wo4")
                        nc.sync.dma_start(wt4[:], oR[:, kb * 4:(kb + 1) * 4, mp * 256:(mp + 1) * 256])
                        for kk in range(4):
                            k = kb * 4 + kk
                            for j in range(2):
                                nc.tensor.matmul(p_o2[j][:], wt4[:, kk, j * 128:(j + 1) * 128], aqt[k][:], start=(k == 0), stop=(k == KH - 1))
                    for j in range(2):
                        m = mp * 2 + j
                        rs = hb.tile([128, TPC], F32, tag="rs")
                        nc.sync.dma_start(rs[:], resid.ap()[m * 128:(m + 1) * 128, :])
                        ht = hb.tile([128, TPC], F32, tag="ht")
                        nc.vector.scalar_tensor_tensor(ht[:], p_o2[j][:], o_scale, rs[:], op0=ALU.mult, op1=ALU.add)
                        nc.sync.dma_start(h_sp.ap()[m * 128:(m + 1) * 128, :], ht[:])
                        z2 = hb.tile([128, TPC], F32, tag="z2")
                        nc.scalar.activation(z2[:], ht[:], AF.Square)
                        nc.tensor.matmul(p_ss[:], ones[:], z2[:], start=(m == 0), stop=(m == KH - 1))
                vt = sm.tile([1, TPC], F32, tag="vt")
                nc.vector.tensor_scalar(vt[:], p_ss[:], 1.0 / H, EPS, op0=ALU.mult, op1=ALU.add)
                yt = _newton_rsqrt(nc, sm, vt, TPC)
                p_bc = ps_bc.tile([128, TPC], F32, tag="pbc")
                nc.tensor.matmul(p_bc[:], ones1[:], yt[:], start=True, stop=True)
                rs_bc = sm.tile([128, TPC], F32, tag="rsbc")
                nc.vector.tensor_copy(rs_bc[:], p_bc[:])
                for m in range(KH):
                    hh = hb.tile([128, TPC], F32, tag="hh")
                    nc.sync.dma_start(hh[:], h_sp.ap()[m * 128:(m + 1) * 128, :])
                    f = hb.tile([128, TPC], F32, tag="f2")
                    nc.vector.scalar_tensor_tensor(f[:], hh[:], rms2_t[:, m:m + 1], rs_bc[:], op0=ALU.mult, op1=ALU.mult)
                    nc.vector.tensor_scalar(f[:], f[:], MAGIC, MAGIC, op0=ALU.add, op1=ALU.subtract)
                    nc.vector.tensor_scalar(ln2q[m][:], f[:], 127.0, -127.0, op0=ALU.min, op1=ALU.max)

            # ---- gate/up (GRP=2, double-buffered PSUM ping-pong) ----
            with contextlib.ExitStack() as ph:
                wst = ph.enter_context(tc.tile_pool(name="wst2", bufs=4))
                eb = ph.enter_context(tc.tile_pool(name="eb", bufs=4))
                ps_g = ph.enter_context(tc.tile_pool(name="ps_g", bufs=2, space="PSUM"))
                ps_u = ph.enter_context(tc.tile_pool(name="ps_u", bufs=2, space="PSUM"))
                GRP = 2
                guR = gu_w.ap().rearrange("(kh p) m -> p kh m", p=128)
                for ig in range(_ceil_div(KI, GRP)):
                    i0 = ig * GRP
                    nI = min(GRP, KI - i0)
                    pgs = [ps_g.tile([128, TPC], F32, tag=f"pg{j}", name=f"pg{j}") for j in range(nI)]
                    pus = [ps_u.tile([128, TPC], F32, tag=f"pu{j}", name=f"pu{j}") for j in range(nI)]
                    for kb in range(KH // 4):
                        guw4 = wst.tile([128, 4, nI * 256], F16, tag="guw4")
                        nc.sync.dma_start(guw4[:], guR[:, kb * 4:(kb + 1) * 4, i0 * 256:(i0 + nI) * 256])
                        for kk in range(4):
                            k = kb * 4 + kk
                            for j in range(nI):
                                nc.tensor.matmul(pgs[j][:], guw4[:, kk, j * 256:j * 256 + 128], ln2q[k][:], start=(k == 0), stop=(k == KH - 1))
                                nc.tensor.matmul(pus[j][:], guw4[:, kk, j * 256 + 128:j * 256 + 256], ln2q[k][:], start=(k == 0), stop=(k == KH - 1))
                    for j in range(nI):
                        sg = eb.tile([128, TPC], F32, tag="sg")
                        nc.scalar.activation(sg[:], pgs[j][:], AF.Silu, scale=gate_scale)
                        f = eb.tile([128, TPC], F32, tag="f3")
                        nc.vector.scalar_tensor_tensor(f[:], pus[j][:], upds, sg[:], op0=ALU.mult, op1=ALU.mult)
                        nc.vector.tensor_scalar(f[:], f[:], MAGIC, MAGIC, op0=ALU.add, op1=ALU.subtract)
                        nc.vector.tensor_scalar(mq[i0 + j][:], f[:], 127.0, -127.0, op0=ALU.min, op1=ALU.max)

            # ---- down + final (GRP=2, double-buffered) ----
            with contextlib.ExitStack() as ph:
                wst = ph.enter_context(tc.tile_pool(name="wst3", bufs=4))
                eb = ph.enter_context(tc.tile_pool(name="eb2", bufs=4))
                ps_d = ph.enter_context(tc.tile_pool(name="ps_d", bufs=2, space="PSUM"))
                GRP = 2
                dR = down_w.ap().rearrange("(ki p) m -> p ki m", p=128)
                for mg in range(_ceil_div(KH, GRP)):
                    m0 = mg * GRP
                    nM = min(GRP, KH - m0)
                    pds = [ps_d.tile([128, TPC], F32, tag=f"pd{j}", name=f"pd{j}") for j in range(nM)]
                    for kb in range(_ceil_div(KI, 4)):
                        k0 = kb * 4
                        nK = min(4, KI - k0)
                        dw4 = wst.tile([128, nK, nM * 128], F16, tag="dw4", name=f"dw4_{mg}_{kb}")
                        nc.sync.dma_start(dw4[:], dR[:, k0:k0 + nK, m0 * 128:(m0 + nM) * 128])
                        for kk in range(nK):
                            k = k0 + kk
                            for j in range(nM):
                                nc.tensor.matmul(pds[j][:], dw4[:, kk, j * 128:(j + 1) * 128], mq[k][:], start=(k == 0), stop=(k == KI - 1))
                    for j in range(nM):
                        m = m0 + j
                        hh = eb.tile([128, TPC], F32, tag="hh2")
                        nc.sync.dma_start(hh[:], h_sp.ap()[m * 128:(m + 1) * 128, :])
                        ot = eb.tile([128, TPC], F32, tag="ot")
                        nc.vector.scalar_tensor_tensor(ot[:], pds[j][:], down_scale, hh[:], op0=ALU.mult, op1=ALU.add)
                        nc.sync.dma_start(out_T.ap()[m * 128:(m + 1) * 128, :], ot[:])
    nc.compile()
    return nc


def _mask_schedule(mask, S, TPC):
    """Classify [qb, kt] tiles of mask^T: 'keep' (all 0), 'skip' (all <=-1e8),
    else index into deduped 0/1 multiplicative-mask array (applied post-exp)."""
    NKT = S // 128
    QPB = S // TPC
    mT = np.ascontiguousarray(mask.T)  # [kt, qt]
    sched = [[None] * NKT for _ in range(QPB)]
    tiles = []
    keys = {}
    for qb in range(QPB):
        for kt in range(NKT):
            sub = mT[kt * 128:(kt + 1) * 128, qb * TPC:(qb + 1) * TPC]
            if np.all(sub == 0):
                sched[qb][kt] = "keep"
            elif np.all(sub <= -1e8):
                sched[qb][kt] = "skip"
            else:
                key = sub.tobytes()
                if key not in keys:
                    keys[key] = len(tiles)
                    tiles.append((sub == 0).astype(np.float32))  # 1 where kept
                sched[qb][kt] = keys[key]
    if not tiles:
        tiles = [np.zeros((128, TPC), np.float32)]
    return sched, np.stack(tiles)


def _rope_tables(S, HD, qkv_scale):
    # Compute cos/sin via jax, mirroring the reference bit-for-bit (XLA's
    # fp32 cos differs from numpy's by ~1e-3 at large args — the quantization
    # steps downstream amplify that to ~1.3e-2 final error).
    import jax.numpy as jnp
    inv_freq = 1.0 / (10000.0 ** (jnp.arange(0, HD, 2, dtype=jnp.float32) / HD))
    freqs = jnp.outer(jnp.arange(S, dtype=jnp.float32), inv_freq)
    emb_j = jnp.concatenate([freqs, freqs], axis=-1)  # [S,HD]
    cos = np.asarray(jnp.cos(emb_j)).T.astype(np.float64)  # [HD, S]
    sin = np.asarray(jnp.sin(emb_j)).T.astype(np.float64)
    hh = HD // 2
    sgn = np.ones((HD, 1))
    sgn[:hh] = -1.0
    # sin table is pre-swapped+signed: row d holds sign(d)*sin[sigma(d)] where
    # sigma swaps halves -- because the kernel multiplies the SHIFTED q by it.
    sin_sw = np.concatenate([sin[hh:], sin[:hh]], axis=0)
    sq = math.sqrt(HD)
    cosq = (cos * qkv_scale / sq).astype(np.float32)
    sinq = (sin_sw * sgn * qkv_scale / sq).astype(np.float32)
    cosk = (cos * qkv_scale).astype(np.float32)
    sink = (sin_sw * sgn * qkv_scale).astype(np.float32)
    return cosq, sinq, cosk, sink


def kernel(hidden_states, attention_mask, rms1_w, rms2_w, qkv_w, o_w, gate_w,
           up_w, down_w, qkv_scale, o_quant_scale, o_scale, gate_scale,
           up_scale, down_quant_scale, down_scale):
    B, S, H = hidden_states.shape
    NH, HD = 32, 128
    I = gate_w.shape[0]
    T = B * S
    TPC = T // NC_
    KH = H // 128
    HPC = NH // NC_
    DPC = HPC * HD
    f16 = np.float16

    hid_T = np.ascontiguousarray(hidden_states.reshape(T, H).T.astype(np.float32))
    sched, maskp = _mask_schedule(np.asarray(attention_mask)[0, 0], S, TPC)
    cosq, sinq, cosk, sink = _rope_tables(S, HD, float(qkv_scale))
    rms1_t = np.ascontiguousarray(np.asarray(rms1_w, np.float32).reshape(KH, 128).T)
    rms2_t = np.ascontiguousarray(np.asarray(rms2_w, np.float32).reshape(KH, 128).T)

    qkv_w = np.asarray(qkv_w)
    prog1 = build_prog1(B, S, H, NH, HD, TPC, sched, maskp.shape[0],
                        float(qkv_scale), float(o_quant_scale))
    in1 = []
    for c in range(NC_):
        qs = qkv_w[c * DPC:(c + 1) * DPC]              # q rows
        ks = qkv_w[H + c * DPC: H + (c + 1) * DPC]     # k rows
        vs = qkv_w[2 * H + c * DPC: 2 * H + (c + 1) * DPC]
        qk_wT = np.ascontiguousarray(np.concatenate([qs, ks], 0).T).astype(f16)
        v_wT = np.ascontiguousarray(vs.T).astype(f16)
        in1.append({
            "hid_T": hid_T, "qk_wT": qk_wT, "v_wT": v_wT,
            "cosq": cosq, "sinq": sinq, "cosk": cosk, "sink": sink,
            "rms1": rms1_t, "maskp": maskp,
        })
    trace = bool(os.environ.get("KTRACE"))
    global LAST_EXEC_NS
    LAST_EXEC_NS = []
    global RES1, RES2
    res1 = run_bass_kernel_spmd(prog1, in1, core_ids=list(range(NC_)), trace=trace)
    RES1 = res1
    if res1.exec_time_ns:
        LAST_EXEC_NS.append(res1.exec_time_ns)
    aq_full = np.concatenate([r["aq_out"] for r in res1.results], axis=0)  # [H, T]

    prog2 = build_prog2(H, I, TPC, float(o_scale), float(gate_scale),
                        float(up_scale) / float(down_quant_scale), float(down_scale))
    o_wT = np.ascontiguousarray(np.asarray(o_w).T).astype(f16)
    gate_wT = np.asarray(gate_w).T.astype(f16)   # [H, I]
    up_wT = np.asarray(up_w).T.astype(f16)       # [H, I]
    KI = I // 128
    # interleave gate/up in 256-col blocks: [g(2 tiles) u(2 tiles)] per 512? no:
    # per 128-col tile j: block j holds gate cols [j*128:(j+1)*128] then up same.
    gu = np.empty((H, 2 * I), f16)
    for j in range(KI):
        gu[:, j * 256:j * 256 + 128] = gate_wT[:, j * 128:(j + 1) * 128]
        gu[:, j * 256 + 128:j * 256 + 256] = up_wT[:, j * 128:(j + 1) * 128]
    gu_wT = np.ascontiguousarray(gu)
    down_wT = np.ascontiguousarray(np.asarray(down_w).T).astype(f16)
    in2 = []
    for c in range(NC_):
        tok = slice(c * TPC, (c + 1) * TPC)
        in2.append({
            "aq_T": np.ascontiguousarray(aq_full[:, tok]),
            "resid_T": np.ascontiguousarray(hid_T[:, tok]),
            "o_wT": o_wT, "gu_wT": gu_wT, "down_wT": down_wT,
            "rms2": rms2_t,
        })
    res2 = run_bass_kernel_spmd(prog2, in2, core_ids=list(range(NC_)), trace=trace)
    RES2 = res2
    if res2.exec_time_ns:
        LAST_EXEC_NS.append(res2.exec_time_ns)
    out_T = np.concatenate([r["out_T"] for r in res2.results], axis=1)  # [H, T]
    return np.ascontiguousarray(out_T.T).reshape(B, S, H).astype(np.float32)
